# revision 39
# baseline (speedup 1.0000x reference)
"""Trainium2 Bass kernel for DynamicPathCrossAttention.

Sharding: batch-parallel — core b computes batch element b end-to-end. The
path-gating MLP is evaluated on the host from the runtime inputs; each core
only computes cross-attention for its batch element's TOP_K=2 selected paths.

Weight folding (host, shared across cores): because the reference chain is
linear around the softmax, adjacent projection pairs collapse:
  logits = Q Wq^T Wk S^T          -> G_p = Wq^T @ Wk_p     (logits = Q G S^T)
  out    = attn S Wv^T Wo^T (...) -> H_p = Wo @ Wv_p       (out = attn S H^T)
so the device never materializes Qp, K, or V — 8 big matmul units per core
instead of 10. Bias algebra: the per-q logit terms cancel inside softmax; the
per-k term ships as an exp() bias column vb = (S @ Wk^T bq) / sqrt(D); bv
folds into an effective output bias boe = bo + sum_p w_p (Wo @ bv_p).

Device pipeline per path (all contractions on SBUF partitions, zero
on-device transposes; all matmuls float32r = full PE rate, ~1e-4 rel err):
  TMP[d',q]   = sum_d  G[d,d'] QT[d,q]        (lhsT=G resident, rhs=QT chunk)
  logitsT[k,q]= sum_d' ST[d',k] TMP[d',q]     (lhsT=ST resident, rhs=TMP)
  expT        = exp(logitsT/sqrt(D) + vb[k])  (ACT from PSUM, bias fused)
  rowsum[1,q] = sum_k expT[k,q]               (ones-matmul)
  AOS[d',q]   = sum_k SN[k,d'] expT[k,q]      (lhsT=SN chunk, rhs=expT)
  AOSs        = AOS * (w_p/rowsum broadcast)  (DVE from PSUM)
  outT[o,q]  += sum_d' HT[d',o] AOSs[d',q]    (lhsT=HT resident; path-0 half
                                               stashed in SBUF, path-1 adds)
"""

import numpy as np

D = 1024
P = 4
TOP_K = 2
B = 8
LQ = 1024
LK = 1024
N_CORES = 8

_CACHE = {}


def _build_program():
    import concourse.bass as bass  # noqa: F401
    import concourse.mybir as mybir
    import concourse.tile as tile
    from concourse import bacc

    f32 = mybir.dt.float32
    f32r = mybir.dt.float32r
    Exp = mybir.ActivationFunctionType.Exp
    Identity = mybir.ActivationFunctionType.Identity
    ADD = mybir.AluOpType.add
    MULT = mybir.AluOpType.mult

    nc = bacc.Bacc(
        "TRN2", target_bir_lowering=False, debug=False, enable_asserts=False
    )

    def din(name, shape):
        return nc.dram_tensor(name, shape, f32, kind="ExternalInput").ap()

    QT = din("QT", [D, LQ])
    ST_d = [din(f"S{p}T", [D, LK]) for p in range(2)]
    SN_d = [din(f"SN{p}", [LK, D]) for p in range(2)]
    G_d = [din(f"G{p}", [D, D]) for p in range(2)]
    HT_d = [din(f"HT{p}", [D, D]) for p in range(2)]
    vb_d = [din(f"vb{p}", [LK, 1]) for p in range(2)]
    boe_c = din("boe", [D, 1])
    wgt = din("wgt", [1, 2])
    ones_col_d = din("ones_col", [128, 1])
    ones_row_d = din("ones_row", [1, 128])
    outT = nc.dram_tensor("outT", [D, LQ], f32, kind="ExternalOutput").ap()

    SCALE = 1.0 / float(np.sqrt(D))
    nD = D // 128

    with tile.TileContext(nc) as tc:
        import contextlib

        with contextlib.ExitStack() as ctx:
            const = ctx.enter_context(tc.tile_pool(name="const", bufs=1))
            stream = ctx.enter_context(tc.tile_pool(name="stream", bufs=10))
            tap = ctx.enter_context(tc.tile_pool(name="tap", bufs=1))
            stp = ctx.enter_context(tc.tile_pool(name="stp", bufs=1))
            kvp = ctx.enter_context(tc.tile_pool(name="kvp", bufs=1))
            expp = ctx.enter_context(tc.tile_pool(name="expp", bufs=1))
            o0p = ctx.enter_context(tc.tile_pool(name="o0p", bufs=1))
            smallp = ctx.enter_context(tc.tile_pool(name="smallp", bufs=2))
            vecp = ctx.enter_context(tc.tile_pool(name="vecp", bufs=1))
            osbp = ctx.enter_context(tc.tile_pool(name="osbp", bufs=4))
            psp = ctx.enter_context(tc.tile_pool(name="psp", bufs=8, space="PSUM"))
            dramp = ctx.enter_context(tc.tile_pool(name="dramp", bufs=2, space="DRAM"))

            # ---- constants (DMAs deferred behind the first compute chunks) --
            ones_col = const.tile([128, 1], f32r)
            vb_t = [const.tile([128, nD], f32, name=f"vb_t{p}") for p in range(2)]
            boe_t = const.tile([128, nD], f32)
            wgt_sb = const.tile([1, 2], f32)

            def emit_const_dmas():
                nc.sync.dma_start(ones_col[:], ones_col_d[:].bitcast(f32r))
                for p in range(2):
                    nc.sync.dma_start(
                        vb_t[p][:], vb_d[p].rearrange("(t p) o -> p (t o)", p=128)
                    )
                nc.sync.dma_start(
                    boe_t[:], boe_c.rearrange("(t p) o -> p (t o)", p=128)
                )
                nc.sync.dma_start(wgt_sb[:], wgt[:])

            def load_st_tile(p, d_t):
                s_tile = stp.tile([128, LK], f32r, tag=f"st{d_t}", name=f"st{d_t}")
                nc.sync.dma_start(
                    s_tile[:],
                    ST_d[p][d_t * 128 : (d_t + 1) * 128, :].bitcast(f32r),
                )
                return s_tile

            out0 = [
                o0p.tile([128, LQ], f32, name=f"out0_{i}") for i in range(nD)
            ]

            for p in range(2):
                # =====================================================
                # TMP[d', q] = sum_d G[d, d'] QT[d, q]
                # G resident in kv slots; QT streams once per path.
                # ST for this path trickles in behind.
                # =====================================================
                g_res = []
                st = []
                tmp_t = [
                    tap.tile([128, LQ], f32r, tag=f"ta{i}", name=f"tmp{i}")
                    for i in range(nD)
                ]
                for q_b in range(2):
                    ps_t = [
                        psp.tile([128, 512], f32, tag="acc", name="ps_t")
                        for _ in range(8)
                    ]
                    for d_t in range(8):
                        qt_ch = stream.tile([128, 512], f32r, tag="wc", name="qtc")
                        nc.sync.dma_start(
                            qt_ch[:],
                            QT[
                                d_t * 128 : (d_t + 1) * 128,
                                q_b * 512 : (q_b + 1) * 512,
                            ].bitcast(f32r),
                        )
                        if q_b == 0:
                            if p == 0 and d_t == 0:
                                # two independently-waitable half tiles so the
                                # first matmuls start on the first 256KB
                                ga = kvp.tile([128, 512], f32r, tag="kv0a", name="g0a")
                                nc.sync.dma_start(
                                    ga[:], G_d[p][0:128, 0:512].bitcast(f32r)
                                )
                                gb = kvp.tile([128, 512], f32r, tag="kv0b", name="g0b")
                                nc.sync.dma_start(
                                    gb[:], G_d[p][0:128, 512:1024].bitcast(f32r)
                                )
                                g_res.append((ga, gb))
                            else:
                                g_tile = kvp.tile(
                                    [128, D], f32r, tag=f"kv{d_t}", name=f"g{d_t}"
                                )
                                nc.sync.dma_start(
                                    g_tile[:],
                                    G_d[p][
                                        d_t * 128 : (d_t + 1) * 128, :
                                    ].bitcast(f32r),
                                )
                                g_res.append(g_tile)
                        if p == 0 and q_b == 0 and d_t == 2:
                            emit_const_dmas()
                        for dp_t in range(8):
                            g = g_res[d_t]
                            if isinstance(g, tuple):
                                lhsT = (
                                    g[0][:, dp_t * 128 : (dp_t + 1) * 128]
                                    if dp_t < 4
                                    else g[1][:, (dp_t - 4) * 128 : (dp_t - 3) * 128]
                                )
                            else:
                                lhsT = g[:, dp_t * 128 : (dp_t + 1) * 128]
                            nc.tensor.matmul(
                                ps_t[dp_t][:],
                                lhsT,
                                qt_ch[:],
                                start=(d_t == 0),
                                stop=(d_t == 7),
                            )
                        # trickle this path's S^T behind the TMP chunks
                        if q_b == 1 and d_t in (0, 2, 4, 6):
                            st.append(load_st_tile(p, len(st)))
                    for dp_t in range(8):
                        dst = tmp_t[dp_t][:, q_b * 512 : (q_b + 1) * 512]
                        if dp_t % 2 == 0:
                            nc.scalar.activation(dst, ps_t[dp_t][:], Identity)
                        else:
                            nc.vector.tensor_copy(dst, ps_t[dp_t][:])
                while len(st) < 8:
                    st.append(load_st_tile(p, len(st)))

                # =====================================================
                # logits + exp + row-sums (both q blocks)
                # =====================================================
                expt = [
                    [
                        expp.tile([128, 512], f32r, tag=f"ex{q_b}_{k_t}", name="expt")
                        for k_t in range(8)
                    ]
                    for q_b in range(2)
                ]
                sbc = [None, None]

                def emit_logits_exp(q_b):
                    for k_t in range(8):
                        ps = psp.tile([128, 512], f32, tag="acc", name="ps_l")
                        for dp_t in range(8):
                            nc.tensor.matmul(
                                ps[:],
                                st[dp_t][:, k_t * 128 : (k_t + 1) * 128],
                                tmp_t[dp_t][:, q_b * 512 : (q_b + 1) * 512],
                                start=(dp_t == 0),
                                stop=(dp_t == 7),
                            )
                        nc.scalar.activation(
                            expt[q_b][k_t][:],
                            ps[:],
                            Exp,
                            bias=vb_t[p][:, k_t : k_t + 1],
                            scale=SCALE,
                        )

                def emit_rowsum(q_b):
                    ps_s = psp.tile([1, 512], f32, tag="acc", name="ps_s")
                    for k_t in range(8):
                        nc.tensor.matmul(
                            ps_s[:],
                            ones_col[:],
                            expt[q_b][k_t][:],
                            start=(k_t == 0),
                            stop=(k_t == 7),
                        )
                    return ps_s

                def emit_sbc(q_b, ps_s):
                    rs = vecp.tile([1, 512], f32, tag="rs", name="rs")
                    nc.vector.reciprocal(rs[:], ps_s[:])
                    s_row = vecp.tile([1, 512], f32, tag="srow", name="s_row")
                    nc.vector.tensor_scalar_mul(s_row[:], rs[:], wgt_sb[0:1, p : p + 1])
                    # broadcast across partitions via a DRAM bounce (the PE
                    # stays out of it; DRAM-source partition_broadcast works)
                    srow_d = dramp.tile([1, 512], f32, tag="srd", name="srow_d")
                    nc.sync.dma_start(srow_d[:], s_row[:])
                    sb_t = smallp.tile([128, 512], f32, tag="sbc", name="sb_t")
                    nc.sync.dma_start(sb_t[:], srow_d[0:1, :].partition_broadcast(128))
                    sbc[q_b] = sb_t

                emit_logits_exp(0)
                ps_s0 = emit_rowsum(0)
                emit_logits_exp(1)
                emit_sbc(0, ps_s0)
                ps_s1 = emit_rowsum(1)
                emit_sbc(1, ps_s1)

                # HT resident: reuse the (now dead) ST slots
                ht_res = []
                for dp_t in range(8):
                    h_tile = stp.tile(
                        [128, D], f32r, tag=f"st{dp_t}", name=f"ht{dp_t}"
                    )
                    nc.sync.dma_start(
                        h_tile[:],
                        HT_d[p][dp_t * 128 : (dp_t + 1) * 128, :].bitcast(f32r),
                    )
                    ht_res.append(h_tile)

                # =====================================================
                # AOS[d', q] = sum_k SN[k, d'] expT[k, q], then scale by
                # sbc = w_p / rowsum  (PSUM -> SBUF fused with the copy)
                # =====================================================
                aoss = [
                    tap.tile([128, LQ], f32r, tag=f"ta{i}", name=f"aoss{i}")
                    for i in range(nD)
                ]
                for dp_h in range(2):
                    ps_a = [
                        [
                            psp.tile([128, 512], f32, tag="acc", name="ps_a")
                            for _ in range(2)
                        ]
                        for _ in range(4)
                    ]
                    for k_t in range(8):
                        snc = stream.tile([128, 512], f32r, tag="wc", name="snc")
                        nc.sync.dma_start(
                            snc[:],
                            SN_d[p][
                                k_t * 128 : (k_t + 1) * 128,
                                dp_h * 512 : (dp_h + 1) * 512,
                            ].bitcast(f32r),
                        )
                        for dp_i in range(4):
                            for q_b in range(2):
                                nc.tensor.matmul(
                                    ps_a[dp_i][q_b][:],
                                    snc[:, dp_i * 128 : (dp_i + 1) * 128],
                                    expt[q_b][k_t][:],
                                    start=(k_t == 0),
                                    stop=(k_t == 7),
                                )
                    for dp_i in range(4):
                        dp_t = dp_h * 4 + dp_i
                        for q_b in range(2):
                            nc.vector.tensor_tensor(
                                aoss[dp_t][:, q_b * 512 : (q_b + 1) * 512],
                                ps_a[dp_i][q_b][:],
                                sbc[q_b][:],
                                MULT,
                            )

                # =====================================================
                # outT[o, q] += sum_d' HT[d', o] AOSs[d', q]
                # path 0 stashes into SBUF (with boe bias); path 1 adds
                # and writes out.  o_t-outer so copy+DMA pipelines.
                # =====================================================
                for q_b in range(2):
                    for o_t in range(8):
                        ps = psp.tile([128, 512], f32, tag="acc", name="ps_o")
                        for dp_t in range(8):
                            nc.tensor.matmul(
                                ps[:],
                                ht_res[dp_t][:, o_t * 128 : (o_t + 1) * 128],
                                aoss[dp_t][:, q_b * 512 : (q_b + 1) * 512],
                                start=(dp_t == 0),
                                stop=(dp_t == 7),
                            )
                        if p == 0:
                            dst = out0[o_t][:, q_b * 512 : (q_b + 1) * 512]
                            if o_t % 2 == 0:
                                nc.scalar.activation(
                                    dst, ps[:], Identity,
                                    bias=boe_t[:, o_t : o_t + 1],
                                )
                            else:
                                nc.vector.tensor_scalar_add(
                                    dst, ps[:], boe_t[:, o_t : o_t + 1]
                                )
                        else:
                            osb = osbp.tile([128, 512], f32, tag="osb", name="osb")
                            if o_t == 7 and q_b == 1:
                                # split the final tile so copy and DMA pipeline
                                for h in range(2):
                                    sl = slice(h * 256, (h + 1) * 256)
                                    nc.vector.tensor_tensor(
                                        osb[:, sl],
                                        ps[:, sl],
                                        out0[o_t][:, q_b * 512 + h * 256 : q_b * 512 + (h + 1) * 256],
                                        ADD,
                                    )
                                    nc.sync.dma_start(
                                        outT[
                                            o_t * 128 : (o_t + 1) * 128,
                                            q_b * 512 + h * 256 : q_b * 512 + (h + 1) * 256,
                                        ],
                                        osb[:, sl],
                                    )
                            else:
                                nc.vector.tensor_tensor(
                                    osb[:],
                                    ps[:],
                                    out0[o_t][:, q_b * 512 : (q_b + 1) * 512],
                                    ADD,
                                )
                                nc.sync.dma_start(
                                    outT[
                                        o_t * 128 : (o_t + 1) * 128,
                                        q_b * 512 : (q_b + 1) * 512,
                                    ],
                                    osb[:],
                                )

    nc.compile()
    return nc


def _get_program():
    if "nc" not in _CACHE:
        _CACHE["nc"] = _build_program()
    return _CACHE["nc"]


def _host_gating(Q, Wq, bq, Wm1, bm1, Wm2, bm2):
    """Replicates the reference path-score MLP + top-k sparse weights."""
    Qm = Q.astype(np.float64).mean(axis=1)  # [B, D]
    pooled = Qm @ Wq.astype(np.float64).T + bq.astype(np.float64)
    h = np.maximum(pooled @ Wm1.astype(np.float64).T + bm1.astype(np.float64), 0.0)
    pl = h @ Wm2.astype(np.float64).T + bm2.astype(np.float64)  # [B, P]
    pl = pl - pl.max(axis=1, keepdims=True)
    e = np.exp(pl)
    scores = e / e.sum(axis=1, keepdims=True)
    idx = np.argsort(-scores, axis=1, kind="stable")[:, :TOP_K]  # [B, 2]
    w = np.take_along_axis(scores, idx, axis=1)
    wn = w / (w.sum(axis=1, keepdims=True) + 1e-8)
    return idx.astype(np.int64), wn.astype(np.float32)


def kernel(**inputs):
    from concourse.bass_utils import run_bass_kernel_spmd

    Q = np.asarray(inputs["Q"], dtype=np.float32)
    src = np.asarray(inputs["src"], dtype=np.float32)
    Wq = np.asarray(inputs["Wq"], dtype=np.float32)
    bq = np.asarray(inputs["bq"], dtype=np.float32)
    Wk = np.asarray(inputs["Wk"], dtype=np.float32)
    bk = np.asarray(inputs["bk"], dtype=np.float32)  # noqa: F841  (cancels in softmax)
    Wv = np.asarray(inputs["Wv"], dtype=np.float32)
    bv = np.asarray(inputs["bv"], dtype=np.float32)
    Wm1 = np.asarray(inputs["Wm1"], dtype=np.float32)
    bm1 = np.asarray(inputs["bm1"], dtype=np.float32)
    Wm2 = np.asarray(inputs["Wm2"], dtype=np.float32)
    bm2 = np.asarray(inputs["bm2"], dtype=np.float32)
    Wo = np.asarray(inputs["Wo"], dtype=np.float32)
    bo = np.asarray(inputs["bo"], dtype=np.float32)

    idx, wn = _host_gating(Q, Wq, bq, Wm1, bm1, Wm2, bm2)
    SCALE = 1.0 / float(np.sqrt(D))

    nc = _get_program()

    # host-folded weights, shared across cores (<=4 selected paths)
    sel = sorted(set(idx.flatten().tolist()))
    WqT = Wq.T
    G = {p: np.ascontiguousarray(WqT @ Wk[p]) for p in sel}
    HT = {p: np.ascontiguousarray((Wo @ Wv[p]).T) for p in sel}
    g2 = {p: Wk[p].T @ bq for p in sel}
    Wobv = {p: Wo @ bv[p] for p in sel}
    ones_col = np.ones((128, 1), np.float32)
    ones_row = np.ones((1, 128), np.float32)

    in_maps = []
    for b in range(B):
        p0, p1 = int(idx[b, 0]), int(idx[b, 1])
        boe = bo + wn[b, 0] * Wobv[p0] + wn[b, 1] * Wobv[p1]
        m = {
            "QT": np.ascontiguousarray(Q[b].T),
            "S0T": np.ascontiguousarray(src[p0, b].T),
            "S1T": np.ascontiguousarray(src[p1, b].T),
            "SN0": np.ascontiguousarray(src[p0, b]),
            "SN1": np.ascontiguousarray(src[p1, b]),
            "G0": G[p0],
            "G1": G[p1],
            "HT0": HT[p0],
            "HT1": HT[p1],
            "vb0": np.ascontiguousarray(
                ((src[p0, b] @ g2[p0]) * SCALE).reshape(LK, 1).astype(np.float32)
            ),
            "vb1": np.ascontiguousarray(
                ((src[p1, b] @ g2[p1]) * SCALE).reshape(LK, 1).astype(np.float32)
            ),
            "boe": np.ascontiguousarray(boe.reshape(D, 1).astype(np.float32)),
            "wgt": np.ascontiguousarray(wn[b].reshape(1, 2)),
            "ones_col": ones_col,
            "ones_row": ones_row,
        }
        in_maps.append(m)

    res = run_bass_kernel_spmd(nc, in_maps, core_ids=list(range(N_CORES)))
    out = np.stack([res.results[b]["outT"].T for b in range(B)], axis=0)
    return np.ascontiguousarray(out).astype(np.float32)
